# revision 6
# baseline (speedup 1.0000x reference)
"""Trainium2 Bass kernel: image -> additive-sinusoid audio encoding.

Math (per batch image b):
  gray = 255 * (w . rgb);  rev = flip(gray, rows);  avg = mean(gray)
  px   = clip(3*rev - 2*avg, 0, 255)
  A    = where(px==0, 0, exp(ln10 * (px/160 - 1.5)))            # [M=64 rows, N=64 cols]
  y[t] = sum_m A[m, col(t)] * sin(W[m]*t*dt + PHI0[m]),  col(t) = min(t//361, 63)
  audio= clip(0.5 + 2048*y, -32768, 32767)                       # [ns=23152]

Kernel strategy: t = n*361 + r  =>  angle = theta[i,n] + beta[i,r] (row flip folded
into the host tables), so  sinmat = sin(theta)cos(beta) + cos(theta)sin(beta) and
the gathered einsum becomes dense matmuls of P/Q = A*sin(theta)/A*cos(theta)
against tiny constant cos/sin(beta) banks. Data-parallel over batch: 8 images per
NeuronCore, layout [128 partitions = (batch-half, image-row), 256 = (b2, col)].
"""

import os

import numpy as np

# ---- problem constants (from the nn.Module definition; input-independent) ----
M = 64
N = 64
FL, FH, FS, T = 80.0, 7600.0, 22050, 1.05
NS = 2 * int(0.5 * FS * T)  # 23152
NUM = NS // N  # 361
RMAX = NS - (N - 1) * NUM  # 409 (last column's sample count)
DT = float(np.float32(1.0 / FS))  # reference rounds dt to f32 (jnp weak typing)
TWO_PI = 2.0 * np.pi
B = 64
N_CORES = 8
B_LOC = B // N_CORES  # 8 images per core
SCALE_SSM = (0.5 / np.sqrt(M)) * 32768.0  # 2048
LN10 = float(np.log(10.0))
EXP_A = LN10 / 160.0
EXP_B = -1.5 * LN10
W0, W1, W2 = 0.2989, 0.5870, 0.1140
C00 = 3.0 * 255.0 * W0  # fold of the 3*255*w0 scale into the gray accumulator
R1 = W1 / W0
R2 = W2 / W0
KAVG = 2.0 * 255.0 / (64.0 * 64.0)  # csx -> 2*avg weighting


def _make_tables():
    # LCG phase bank (faithful port, ir starts at 0)
    ia, ic, im = 9301, 49297, 233280
    ir = 0
    phi = []
    for _ in range(M):
        ir = (ir * ia + ic) % im
        phi.append(TWO_PI * ir / im)
    phi32 = np.array(phi, np.float64).astype(np.float32)
    w32 = (TWO_PI * FL * (FH / FL) ** (np.arange(M) / (M - 1))).astype(np.float32)

    # fold the row flip (tf.reverse on axis 1) into the tables: row i uses W[63-i]
    wf = w32[::-1].astype(np.float64)
    phif = phi32[::-1].astype(np.float64)

    n_idx = np.arange(N, dtype=np.float64)
    theta = wf[:, None] * (n_idx[None, :] * NUM * DT) + phif[:, None]  # [64, 64]
    st = np.sin(theta)
    ct = np.cos(theta)

    r_idx = np.arange(RMAX, dtype=np.float64)
    beta = wf[:, None] * (r_idx[None, :] * DT)  # [64, 409]
    cb = np.tile((SCALE_SSM * np.cos(beta)).astype(np.float32), (2, 1))  # [128, RMAX]
    sb = np.tile((SCALE_SSM * np.sin(beta)).astype(np.float32), (2, 1))

    # [p=(bh,i), (b2,n)] broadcast of the theta tables
    stbc = np.tile(st[None, :, None, :], (2, 1, 4, 1)).reshape(128, 256)
    ctbc = np.tile(ct[None, :, None, :], (2, 1, 4, 1)).reshape(128, 256)

    halfsel = np.zeros((128, 2), np.float32)
    halfsel[:64, 0] = 1.0
    halfsel[64:, 1] = 1.0
    halfones2 = np.zeros((2, 128), np.float32)
    halfones2[0, :64] = 1.0
    halfones2[1, 64:] = 1.0

    wrep = np.tile(
        (np.array([W0, W1, W2], np.float64) * KAVG).astype(np.float32), 256
    ).reshape(1, 768)
    wrep = np.broadcast_to(wrep, (2, 768)).copy()

    return dict(
        stbc=stbc.astype(np.float32),
        ctbc=ctbc.astype(np.float32),
        cb=cb,
        sb=sb,
        halfsel=halfsel,
        halfones2=halfones2,
        wrep=wrep,
    )


_TABLES = None


def tables():
    global _TABLES
    if _TABLES is None:
        _TABLES = _make_tables()
    return _TABLES


def build_nc():
    import concourse.bacc as bacc
    import concourse.bass as bass
    import concourse.mybir as mybir
    import concourse.tile as tile

    f32 = mybir.dt.float32
    Alu = mybir.AluOpType
    Act = mybir.ActivationFunctionType

    nc = bacc.Bacc("TRN2", target_bir_lowering=False, debug=False, num_devices=N_CORES)

    x_d = nc.dram_tensor("x", [B_LOC, 64, 64, 3], f32, kind="ExternalInput")
    stbc_d = nc.dram_tensor("stbc", [128, 256], f32, kind="ExternalInput")
    ctbc_d = nc.dram_tensor("ctbc", [128, 256], f32, kind="ExternalInput")
    cb_d = nc.dram_tensor("cb", [128, RMAX], f32, kind="ExternalInput")
    sb_d = nc.dram_tensor("sb", [128, RMAX], f32, kind="ExternalInput")
    halfsel_d = nc.dram_tensor("halfsel", [128, 2], f32, kind="ExternalInput")
    halfones2_d = nc.dram_tensor("halfones2", [2, 128], f32, kind="ExternalInput")
    wrep_d = nc.dram_tensor("wrep", [2, 768], f32, kind="ExternalInput")
    audio_d = nc.dram_tensor("audio", [B_LOC, NS], f32, kind="ExternalOutput")

    af = audio_d[:].rearrange("b t -> (b t)")

    with tile.TileContext(nc) as tc:
        with (
            tc.tile_pool(name="consts", bufs=1) as consts,
            tc.tile_pool(name="work", bufs=1) as work,
            tc.tile_pool(name="outp", bufs=4) as outp,
            tc.tile_pool(name="psum_y", bufs=4, space=bass.MemorySpace.PSUM) as psum_y,
            tc.tile_pool(name="psum_m", bufs=1, space=bass.MemorySpace.PSUM) as psum_m,
        ):
            # ---- constant tables ----
            stbc = consts.tile([128, 256], f32)
            ctbc = consts.tile([128, 256], f32)
            cb = consts.tile([128, RMAX], f32)
            sbt = consts.tile([128, RMAX], f32)
            halfsel = consts.tile([128, 2], f32)
            halfones2 = consts.tile([2, 128], f32)
            wrep = consts.tile([2, 768], f32)
            nc.sync.dma_start(out=stbc, in_=stbc_d[:])
            nc.sync.dma_start(out=ctbc, in_=ctbc_d[:])
            nc.sync.dma_start(out=cb, in_=cb_d[:])
            nc.sync.dma_start(out=sbt, in_=sb_d[:])
            nc.sync.dma_start(out=halfsel, in_=halfsel_d[:])
            nc.sync.dma_start(out=halfones2, in_=halfones2_d[:])
            nc.sync.dma_start(out=wrep, in_=wrep_d[:])

            # ---- input image tile: [p=(bh,i), (b2, j, c)] ----
            X = work.tile([128, 768], f32)
            xv = x_d[:].rearrange("(bh b2) i j c -> bh i b2 j c", bh=2)
            Xv = X[:].rearrange("(bh i) (b2 j c) -> bh i b2 j c", bh=2, b2=4, c=3)
            for bh in range(2):
                nc.sync.dma_start(out=Xv[bh], in_=xv[bh])

            # ---- grayscale accumulate: t = R + r1*G + r2*B  (x255*3*w0 later) ----
            Xc = X[:].rearrange("p (q c) -> p q c", c=3)
            t = work.tile([128, 256], f32)
            nc.vector.scalar_tensor_tensor(
                out=t, in0=Xc[:, :, 1], scalar=float(R1), in1=Xc[:, :, 0],
                op0=Alu.mult, op1=Alu.add,
            )
            nc.vector.scalar_tensor_tensor(
                out=t, in0=Xc[:, :, 2], scalar=float(R2), in1=t,
                op0=Alu.mult, op1=Alu.add,
            )

            # ---- per-image mean path (raw x -> csx -> weighted sum -> bcast) ----
            csxA = psum_m.tile([2, 384], f32)
            csxB = psum_m.tile([2, 384], f32)
            nc.tensor.matmul(csxA, halfsel, X[:, 0:384], start=True, stop=True)
            nc.tensor.matmul(csxB, halfsel, X[:, 384:768], start=True, stop=True)
            csW = work.tile([2, 768], f32)
            nc.vector.tensor_mul(out=csW[:, 0:384], in0=csxA, in1=wrep[:, 0:384])
            nc.vector.tensor_mul(out=csW[:, 384:768], in0=csxB, in1=wrep[:, 384:768])
            csS = work.tile([2, 4, 1], f32)
            nc.vector.reduce_sum(
                out=csS,
                in_=csW[:].rearrange("p (b2 jc) -> p b2 jc", b2=4),
                axis=mybir.AxisListType.X,
            )
            # broadcast per-image 2*avg across the n columns: [2,4,1] -> [2,4,64]
            Rb = work.tile([2, 4, 64], f32)
            nc.vector.tensor_copy(out=Rb, in_=csS.broadcast_to([2, 4, 64]))
            avgbc = psum_m.tile([128, 256], f32)
            nc.tensor.matmul(
                avgbc, halfones2, Rb[:].rearrange("p a b -> p (a b)"),
                start=True, stop=True,
            )

            # ---- px = clip(3*gray - 2*avg, 0, 255) ----
            px = work.tile([128, 256], f32)
            nc.vector.scalar_tensor_tensor(
                out=px, in0=t, scalar=float(C00), in1=avgbc,
                op0=Alu.mult, op1=Alu.subtract,
            )
            nc.vector.tensor_scalar(
                out=px, in0=px, scalar1=0.0, scalar2=255.0,
                op0=Alu.max, op1=Alu.min,
            )

            # ---- A = (px > 0) * exp(EXP_A*px + EXP_B) ----
            expb = consts.tile([128, 1], f32)
            nc.vector.memset(expb, float(EXP_B))
            halfb = consts.tile([128, 1], f32)
            nc.vector.memset(halfb, 0.5)
            E = work.tile([128, 256], f32)
            nc.scalar.activation(
                out=E, in_=px, func=Act.Exp, bias=expb, scale=float(EXP_A)
            )
            A = work.tile([128, 256], f32)
            nc.vector.scalar_tensor_tensor(
                out=A, in0=px, scalar=0.0, in1=E, op0=Alu.is_gt, op1=Alu.mult
            )

            # ---- P = A*sin(theta), Q = A*cos(theta) ----
            P = work.tile([128, 256], f32)
            Q = work.tile([128, 256], f32)
            nc.vector.tensor_mul(out=P, in0=A, in1=stbc)
            nc.vector.tensor_mul(out=Q, in0=A, in1=ctbc)

            # ---- per 2-batch group: Y = P^T@CB + Q^T@SB ; clip ; DMA out ----
            for g in range(4):
                bh, s = divmod(g, 2)
                prt = slice(64 * bh, 64 * (bh + 1))
                col = slice(128 * s, 128 * (s + 1))
                y_ps = psum_y.tile([128, RMAX], f32, tag="y")
                nc.tensor.matmul(y_ps, P[prt, col], cb[prt], start=True, stop=False)
                nc.tensor.matmul(y_ps, Q[prt, col], sbt[prt], start=False, stop=True)

                u = outp.tile([128, RMAX], f32, tag="u")
                nc.scalar.activation(
                    out=u, in_=y_ps, func=Act.Identity, bias=halfb, scale=1.0
                )
                nc.vector.tensor_scalar(
                    out=u, in0=u, scalar1=-32768.0, scalar2=32767.0,
                    op0=Alu.max, op1=Alu.min,
                )

                b0 = 4 * bh + 2 * s
                for half in range(2):
                    bglob = b0 + half
                    base = bglob * NS
                    nc.sync.dma_start(
                        out=af[base : base + (N - 1) * NUM].rearrange(
                            "(n r) -> n r", r=NUM
                        ),
                        in_=u[64 * half : 64 * half + 63, 0:NUM],
                    )
                    nc.sync.dma_start(
                        out=af[base + (N - 1) * NUM : base + NS].rearrange(
                            "(o r) -> o r", o=1
                        ),
                        in_=u[64 * half + 63 : 64 * half + 64, 0:RMAX],
                    )

    nc.compile()
    return nc


_NC = None


def _get_nc():
    global _NC
    if _NC is None:
        _NC = build_nc()
    return _NC


LAST_RESULTS = None


def kernel(x: np.ndarray) -> np.ndarray:
    from concourse.bass_utils import run_bass_kernel_spmd

    x = np.ascontiguousarray(np.asarray(x, dtype=np.float32))
    assert x.shape == (B, 64, 64, 3), x.shape

    nc = _get_nc()
    tbl = tables()
    in_maps = []
    for c in range(N_CORES):
        m = {"x": np.ascontiguousarray(x[c * B_LOC : (c + 1) * B_LOC])}
        m.update(tbl)
        in_maps.append(m)

    trace = os.environ.get("BASS_KERNEL_TRACE", "0") == "1"
    res = run_bass_kernel_spmd(
        nc, in_maps, core_ids=list(range(N_CORES)), trace=trace
    )
    global LAST_RESULTS
    LAST_RESULTS = res
    return np.concatenate([r["audio"] for r in res.results], axis=0)
